# revision 1
# baseline (speedup 1.0000x reference)
"""GNN message-passing (scatter_mean -> BN -> Linear -> ReLU) on 8 TRN2 cores.

Strategy (edge partition via target-node bin-packing):
  - Host bin-packs the 50000 target nodes into 392 groups of 128 slots so
    every group has ~equal total in-degree (~2041 edges).  Core i owns 49
    groups.  Every core runs the identical instruction stream.
  - Device, per group: gather the 16x128 source rows (indirect DMA, one
    128-row call per tile), build a one-hot(target-slot) matrix on DVE, and
    accumulate sum_T[c, n] on the PE via matmul(lhsT=msgs, rhs=onehot).
    Scatter-mean division uses host-precomputed 1/deg broadcast via a K=1
    matmul (ones x recip).
  - BN batch stats: per-core partial sum / sum-of-squares per channel,
    AllReduce'd across the 8 cores (2x64 floats), then folded into the
    Linear: out = relu(agg @ (a*W^T) + b2).
  - Host reassembles the full [50000, 64] output from the per-core bands.
"""

import sys
import heapq

import numpy as np

for _p in ("/opt/trn_rl_repo",):
    if _p not in sys.path:
        sys.path.append(_p)

import concourse.bacc as bacc
import concourse.bass as bass
import concourse.tile as tile
import concourse.mybir as mybir
from concourse import bass_utils

N_NODES = 50000
N_EDGES = 800000
C = 64
BN_EPS = 1e-5
N_CORES = 8


def plan_shard(targets, n_nodes, n_cores, groups_per_core, tiles_per_group):
    """Bin-pack nodes into (n_cores*groups_per_core) groups of 128 slots with
    ~equal total degree. Returns node->(group, slot) and group loads."""
    n_groups = n_cores * groups_per_core
    deg = np.bincount(targets, minlength=n_nodes).astype(np.int64)
    order = np.argsort(-deg, kind="stable")
    node_group = np.empty(n_nodes, np.int32)
    node_slot = np.empty(n_nodes, np.int32)
    heap = [(0, g) for g in range(n_groups)]
    heapq.heapify(heap)
    fill = np.zeros(n_groups, np.int32)
    loads = np.zeros(n_groups, np.int64)
    for n in order:
        d = int(deg[n])
        while True:
            load, g = heapq.heappop(heap)
            if fill[g] < 128:
                break
        node_group[n] = g
        node_slot[n] = fill[g]
        fill[g] += 1
        loads[g] = load + d
        if fill[g] < 128:
            heapq.heappush(heap, (load + d, g))
    cap = tiles_per_group * 128
    if loads.max() > cap:
        raise RuntimeError(f"bin packing overflow: {loads.max()} > {cap}")
    return deg, node_group, node_slot, loads


def build_tables(x, sources, targets, n_nodes, n_cores, gpc, tpg):
    """Build per-core device input tables."""
    deg, node_group, node_slot, _ = plan_shard(targets, n_nodes, n_cores, gpc, tpg)
    n_groups = n_cores * gpc
    cols = gpc * tpg

    eg = node_group[targets]  # group of each edge
    order = np.argsort(eg, kind="stable")
    eg_sorted = eg[order]
    src_sorted = sources[order].astype(np.int32)
    tslot_sorted = node_slot[targets[order]].astype(np.float32)
    gstart = np.searchsorted(eg_sorted, np.arange(n_groups))
    pos = np.arange(len(order)) - gstart[eg_sorted]

    idx_tbl = np.zeros((n_cores, 128, cols), np.int32)
    tgt_tbl = np.full((n_cores, 128, cols), -1.0, np.float32)
    core_of = eg_sorted // gpc
    g_local = eg_sorted % gpc
    tcol = g_local * tpg + pos // 128
    p = pos % 128
    idx_tbl[core_of, p, tcol] = src_sorted
    tgt_tbl[core_of, p, tcol] = tslot_sorted

    recip = (1.0 / np.maximum(deg, 1)).astype(np.float32)
    recip_tbl = np.ones((n_cores, 1, gpc * 128), np.float32)
    nodes = np.arange(n_nodes)
    ncore = node_group[nodes] // gpc
    npos = (node_group[nodes] % gpc) * 128 + node_slot[nodes]
    recip_tbl[ncore, 0, npos] = recip
    recip_tbl = np.tile(recip_tbl, (1, 64, 1))

    return idx_tbl, tgt_tbl, recip_tbl, node_group, node_slot


def build_nc(n_nodes_real, n_nodes_tab, gpc, tpg):
    """Build the SPMD bass program (identical on all cores)."""
    f32 = mybir.dt.float32
    nc = bacc.Bacc("TRN2", num_devices=N_CORES)
    cols = gpc * tpg
    band = gpc * 128

    x_t = nc.dram_tensor("x", [n_nodes_tab, C], f32, kind="ExternalInput")
    idx_t = nc.dram_tensor("idx", [128, cols], mybir.dt.int32, kind="ExternalInput")
    tgt_t = nc.dram_tensor("tgt", [128, cols], f32, kind="ExternalInput")
    recip_t = nc.dram_tensor("recip", [64, band], f32, kind="ExternalInput")
    iota_t = nc.dram_tensor("iota", [128, 128], f32, kind="ExternalInput")
    ones_t = nc.dram_tensor("ones", [1, 128], f32, kind="ExternalInput")
    gamma_t = nc.dram_tensor("gamma", [64, 1], f32, kind="ExternalInput")
    beta_t = nc.dram_tensor("beta", [64, 1], f32, kind="ExternalInput")
    bvec_t = nc.dram_tensor("bvec", [1, 64], f32, kind="ExternalInput")
    wt_t = nc.dram_tensor("wt", [64, 64], f32, kind="ExternalInput")
    y_t = nc.dram_tensor("y", [band, C], f32, kind="ExternalOutput")

    cc_in = nc.dram_tensor("cc_in", [2, 64], f32, kind="Internal")
    cc_out = nc.dram_tensor("cc_out", [2, 64], f32, kind="Internal", addr_space="Shared")

    eq = mybir.AluOpType.is_equal
    with tile.TileContext(nc) as tc:
        with (
            tc.tile_pool(name="const", bufs=1) as cp,
            tc.tile_pool(name="tbl", bufs=1) as tp,
            tc.tile_pool(name="dst", bufs=12) as dp,
            tc.tile_pool(name="oh", bufs=6) as ohp,
            tc.tile_pool(name="agg", bufs=1) as aggp,
            tc.tile_pool(name="sq", bufs=4) as sqp,
            tc.tile_pool(name="st", bufs=1) as stp,
            tc.tile_pool(name="out", bufs=4) as outp,
            tc.tile_pool(name="pg", bufs=2, space="PSUM") as pgp,
            tc.tile_pool(name="po", bufs=2, space="PSUM") as pop,
            tc.tile_pool(name="pb2", bufs=1, space="PSUM") as pb2p,
        ):
            iota_sb = cp.tile([128, 128], f32)
            nc.sync.dma_start(iota_sb[:], iota_t.ap())
            ones_sb = cp.tile([1, 128], f32)
            nc.sync.dma_start(ones_sb[:], ones_t.ap())
            gamma_sb = cp.tile([64, 1], f32)
            nc.sync.dma_start(gamma_sb[:], gamma_t.ap())
            beta_sb = cp.tile([64, 1], f32)
            nc.sync.dma_start(beta_sb[:], beta_t.ap())
            bvec_sb = cp.tile([1, 64], f32)
            nc.sync.dma_start(bvec_sb[:], bvec_t.ap())
            wt_sb = cp.tile([64, 64], f32)
            nc.sync.dma_start(wt_sb[:], wt_t.ap())
            recip_sb = cp.tile([64, band], f32)
            nc.sync.dma_start(recip_sb[:], recip_t.ap())
            idx_sb = tp.tile([128, cols], mybir.dt.int32)
            nc.sync.dma_start(idx_sb[:], idx_t.ap())
            tgt_sb = tp.tile([128, cols], f32)
            nc.sync.dma_start(tgt_sb[:], tgt_t.ap())

            aggT = aggp.tile([64, band], f32)
            sqpart = stp.tile([64, gpc], f32)

            # phase 1: per group, gather + one-hot matmul accumulate
            for g in range(gpc):
                psum_g = pgp.tile([64, 128], f32, tag="pg")
                for t in range(tpg):
                    col = g * tpg + t
                    dst = dp.tile([128, C], f32, tag="dst")
                    nc.gpsimd.indirect_dma_start(
                        out=dst[:],
                        out_offset=None,
                        in_=x_t.ap(),
                        in_offset=bass.IndirectOffsetOnAxis(
                            ap=idx_sb[:, col : col + 1], axis=0
                        ),
                    )
                    oh = ohp.tile([128, 128], f32, tag="oh")
                    nc.vector.tensor_scalar(
                        out=oh[:],
                        in0=iota_sb[:],
                        scalar1=tgt_sb[:, col : col + 1],
                        scalar2=None,
                        op0=eq,
                    )
                    nc.tensor.matmul(
                        out=psum_g[:],
                        lhsT=dst[:],
                        rhs=oh[:],
                        start=(t == 0),
                        stop=(t == tpg - 1),
                    )
                nc.vector.tensor_tensor(
                    out=aggT[:, g * 128 : (g + 1) * 128],
                    in0=psum_g[:],
                    in1=recip_sb[:, g * 128 : (g + 1) * 128],
                    op=mybir.AluOpType.mult,
                )
                sq_scr = sqp.tile([64, 128], f32, tag="sq")
                nc.scalar.activation(
                    out=sq_scr[:],
                    in_=aggT[:, g * 128 : (g + 1) * 128],
                    func=mybir.ActivationFunctionType.Square,
                    accum_out=sqpart[:, g : g + 1],
                )

            # BN partial stats -> collective
            s_col = stp.tile([64, 1], f32)
            nc.vector.tensor_reduce(
                out=s_col[:], in_=aggT[:], axis=mybir.AxisListType.X,
                op=mybir.AluOpType.add,
            )
            q_col = stp.tile([64, 1], f32)
            nc.vector.tensor_reduce(
                out=q_col[:], in_=sqpart[:], axis=mybir.AxisListType.X,
                op=mybir.AluOpType.add,
            )
            nc.sync.dma_start(cc_in.ap()[0:1, :], s_col[:, 0:1])
            nc.sync.dma_start(cc_in.ap()[1:2, :], q_col[:, 0:1])
            nc.gpsimd.collective_compute(
                "AllReduce",
                mybir.AluOpType.add,
                ins=[cc_in.ap()],
                outs=[cc_out.ap()],
                replica_groups=[list(range(N_CORES))],
            )
            ssum = stp.tile([64, 1], f32)
            nc.sync.dma_start(ssum[:], cc_out.ap()[0:1, :])
            qsum = stp.tile([64, 1], f32)
            nc.sync.dma_start(qsum[:], cc_out.ap()[1:2, :])

            # BN constants + fold into linear
            inv_n = 1.0 / float(n_nodes_real)
            mean = stp.tile([64, 1], f32)
            nc.vector.tensor_scalar(
                out=mean[:], in0=ssum[:], scalar1=inv_n, scalar2=None,
                op0=mybir.AluOpType.mult,
            )
            e2 = stp.tile([64, 1], f32)
            nc.vector.tensor_scalar(
                out=e2[:], in0=qsum[:], scalar1=inv_n, scalar2=None,
                op0=mybir.AluOpType.mult,
            )
            m2 = stp.tile([64, 1], f32)
            nc.vector.tensor_tensor(
                out=m2[:], in0=mean[:], in1=mean[:], op=mybir.AluOpType.mult
            )
            var = stp.tile([64, 1], f32)
            nc.vector.tensor_tensor(
                out=var[:], in0=e2[:], in1=m2[:], op=mybir.AluOpType.subtract
            )
            vare = stp.tile([64, 1], f32)
            nc.vector.tensor_scalar(
                out=vare[:], in0=var[:], scalar1=BN_EPS, scalar2=None,
                op0=mybir.AluOpType.add,
            )
            sd = stp.tile([64, 1], f32)
            nc.scalar.activation(
                out=sd[:], in_=vare[:], func=mybir.ActivationFunctionType.Sqrt
            )
            rstd = stp.tile([64, 1], f32)
            nc.vector.reciprocal(out=rstd[:], in_=sd[:])
            a_col = stp.tile([64, 1], f32)
            nc.vector.tensor_tensor(
                out=a_col[:], in0=rstd[:], in1=gamma_sb[:], op=mybir.AluOpType.mult
            )
            w2 = stp.tile([64, 64], f32)
            nc.vector.tensor_scalar(
                out=w2[:], in0=wt_sb[:], scalar1=a_col[:, 0:1], scalar2=None,
                op0=mybir.AluOpType.mult,
            )
            ma = stp.tile([64, 1], f32)
            nc.vector.tensor_tensor(
                out=ma[:], in0=mean[:], in1=a_col[:], op=mybir.AluOpType.mult
            )
            cvec = stp.tile([64, 1], f32)
            nc.vector.tensor_tensor(
                out=cvec[:], in0=beta_sb[:], in1=ma[:], op=mybir.AluOpType.subtract
            )
            pb2 = pb2p.tile([1, 64], f32)
            nc.tensor.matmul(out=pb2[:], lhsT=cvec[:], rhs=wt_sb[:], start=True, stop=True)
            b2 = stp.tile([1, 64], f32)
            nc.vector.tensor_tensor(
                out=b2[:], in0=pb2[:], in1=bvec_sb[:], op=mybir.AluOpType.add
            )

            # phase 2: out = relu(aggT.T @ W2 + b2)
            for g in range(gpc):
                po = pop.tile([128, 64], f32, tag="po")
                nc.tensor.matmul(
                    out=po[:],
                    lhsT=aggT[:, g * 128 : (g + 1) * 128],
                    rhs=w2[:],
                    start=True,
                    stop=False,
                )
                nc.tensor.matmul(
                    out=po[:], lhsT=ones_sb[:], rhs=b2[:], start=False, stop=True
                )
                ot = outp.tile([128, C], f32, tag="ot")
                nc.scalar.activation(
                    out=ot[:], in_=po[:], func=mybir.ActivationFunctionType.Relu
                )
                nc.sync.dma_start(y_t.ap()[g * 128 : (g + 1) * 128, :], ot[:])

    nc.compile()
    return nc


_CACHE = {}


def _get_nc(n_nodes_real, n_nodes_tab, gpc, tpg):
    key = (n_nodes_real, n_nodes_tab, gpc, tpg)
    if key not in _CACHE:
        _CACHE[key] = build_nc(*key)
    return _CACHE[key]


def kernel(x, sources, targets, gamma, beta, W, b, _trace=False):
    return _run(x, sources, targets, gamma, beta, W, b, 49, 16, _trace)


def _run(x, sources, targets, gamma, beta, W, b, gpc, tpg, _trace=False):
    x = np.asarray(x, np.float32)
    sources = np.asarray(sources).astype(np.int32)
    targets = np.asarray(targets).astype(np.int32)
    gamma = np.asarray(gamma, np.float32)
    beta = np.asarray(beta, np.float32)
    W = np.asarray(W, np.float32)
    b = np.asarray(b, np.float32)

    n_nodes = x.shape[0]
    idx_tbl, tgt_tbl, recip_tbl, node_group, node_slot = build_tables(
        x, sources, targets, n_nodes, N_CORES, gpc, tpg
    )

    iota = np.tile(np.arange(128, dtype=np.float32)[None, :], (128, 1))
    ones = np.ones((1, 128), np.float32)
    in_maps = []
    for i in range(N_CORES):
        in_maps.append(
            {
                "x": x,
                "idx": idx_tbl[i],
                "tgt": tgt_tbl[i],
                "recip": recip_tbl[i],
                "iota": iota,
                "ones": ones,
                "gamma": gamma.reshape(64, 1),
                "beta": beta.reshape(64, 1),
                "bvec": b.reshape(1, 64),
                "wt": np.ascontiguousarray(W.T),
            }
        )

    nc = _get_nc(n_nodes, n_nodes, gpc, tpg)
    res = bass_utils.run_bass_kernel_spmd(
        nc, in_maps, core_ids=list(range(N_CORES)), trace=_trace
    )

    out = np.empty((n_nodes, C), np.float32)
    nodes = np.arange(n_nodes)
    ncore = node_group // gpc
    npos = (node_group % gpc) * 128 + node_slot
    for i in range(N_CORES):
        sel = ncore == i
        out[nodes[sel]] = res.results[i]["y"][npos[sel]]
    kernel.last_exec_time_ns = res.exec_time_ns
    return out



# revision 9
# speedup vs baseline: 1.0202x; 1.0202x over previous
"""GNN message-passing (scatter_mean -> BN -> Linear -> ReLU) on 8 TRN2 cores.

Strategy (edge partition via target-node bin-packing):
  - Host bin-packs the 50000 target nodes into 392 groups of 128 slots so
    every group has ~equal total in-degree (~2041 edges).  Core i owns 49
    groups.  Every core runs the identical instruction stream.
  - Device, per group: gather the 16x128 source rows (indirect DMA, one
    128-row call per tile), build a one-hot(target-slot) matrix on DVE, and
    accumulate sum_T[c, n] on the PE via matmul(lhsT=msgs, rhs=onehot).
    Scatter-mean division uses host-precomputed 1/deg broadcast via a K=1
    matmul (ones x recip).
  - BN batch stats: per-core partial sum / sum-of-squares per channel,
    AllReduce'd across the 8 cores (2x64 floats), then folded into the
    Linear: out = relu(agg @ (a*W^T) + b2).
  - Host reassembles the full [50000, 64] output from the per-core bands.
"""

import sys
import heapq

import numpy as np
import ml_dtypes

for _p in ("/opt/trn_rl_repo",):
    if _p not in sys.path:
        sys.path.append(_p)

import concourse.bacc as bacc
import concourse.bass as bass
import concourse.tile as tile
import concourse.mybir as mybir
from concourse import bass_utils

N_NODES = 50000
N_EDGES = 800000
C = 64
BN_EPS = 1e-5
N_CORES = 8


def plan_shard(targets, n_nodes, n_cores, groups_per_core, tiles_per_group):
    """Bin-pack nodes into (n_cores*groups_per_core) groups of 128 slots with
    ~equal total degree. Returns node->(group, slot) and group loads."""
    n_groups = n_cores * groups_per_core
    deg = np.bincount(targets, minlength=n_nodes).astype(np.int64)
    order = np.argsort(-deg, kind="stable")
    node_group = np.empty(n_nodes, np.int32)
    node_slot = np.empty(n_nodes, np.int32)
    heap = [(0, g) for g in range(n_groups)]
    heapq.heapify(heap)
    fill = np.zeros(n_groups, np.int32)
    loads = np.zeros(n_groups, np.int64)
    for n in order:
        d = int(deg[n])
        while True:
            load, g = heapq.heappop(heap)
            if fill[g] < 128:
                break
        node_group[n] = g
        node_slot[n] = fill[g]
        fill[g] += 1
        loads[g] = load + d
        if fill[g] < 128:
            heapq.heappush(heap, (load + d, g))
    cap = tiles_per_group * 128
    if loads.max() > cap:
        raise RuntimeError(f"bin packing overflow: {loads.max()} > {cap}")
    return deg, node_group, node_slot, loads


def build_tables(x, sources, targets, n_nodes, n_cores, gpc, tpg):
    """Build per-core device input tables."""
    deg, node_group, node_slot, _ = plan_shard(targets, n_nodes, n_cores, gpc, tpg)
    n_groups = n_cores * gpc
    cols = gpc * tpg

    eg = node_group[targets]  # group of each edge
    order = np.argsort(eg, kind="stable")
    eg_sorted = eg[order]
    src_sorted = sources[order].astype(np.int32)
    tslot_sorted = node_slot[targets[order]].astype(np.float32)
    gstart = np.searchsorted(eg_sorted, np.arange(n_groups))
    pos = np.arange(len(order)) - gstart[eg_sorted]

    idx_tbl = np.zeros((n_cores, 128, cols), np.int32)
    tgt_tbl = np.full((n_cores, 128, cols), -1.0, np.float32)
    core_of = eg_sorted // gpc
    g_local = eg_sorted % gpc
    tcol = g_local * tpg + pos // 128
    p = pos % 128
    idx_tbl[core_of, p, tcol] = src_sorted
    tgt_tbl[core_of, p, tcol] = tslot_sorted

    recip = (1.0 / np.maximum(deg, 1)).astype(np.float32)
    recip_tbl = np.ones((n_cores, 1, gpc * 128), np.float32)
    nodes = np.arange(n_nodes)
    ncore = node_group[nodes] // gpc
    npos = (node_group[nodes] % gpc) * 128 + node_slot[nodes]
    recip_tbl[ncore, 0, npos] = recip
    recip_tbl = np.tile(recip_tbl, (1, 64, 1))

    return idx_tbl, tgt_tbl, recip_tbl, node_group, node_slot


def build_nc(n_nodes_real, n_nodes_tab, gpc, tpg):
    """Build the SPMD bass program (identical on all cores)."""
    f32 = mybir.dt.float32
    bf16 = mybir.dt.bfloat16
    nc = bacc.Bacc("TRN2", num_devices=N_CORES)
    cols = gpc * tpg
    band = gpc * 128

    x_t = nc.dram_tensor("x", [n_nodes_tab, C], bf16, kind="ExternalInput")
    idx_t = nc.dram_tensor("idx", [128, cols], mybir.dt.int32, kind="ExternalInput")
    tgt_t = nc.dram_tensor("tgt", [128, cols], f32, kind="ExternalInput")
    recip_t = nc.dram_tensor("recip", [64, band], f32, kind="ExternalInput")
    iota_t = nc.dram_tensor("iota", [128, 128], bf16, kind="ExternalInput")
    ones_t = nc.dram_tensor("ones", [1, 128], bf16, kind="ExternalInput")
    gamma_t = nc.dram_tensor("gamma", [64, 1], f32, kind="ExternalInput")
    beta_t = nc.dram_tensor("beta", [64, 1], f32, kind="ExternalInput")
    bvec_t = nc.dram_tensor("bvec", [1, 64], f32, kind="ExternalInput")
    wt_t = nc.dram_tensor("wt", [64, 64], f32, kind="ExternalInput")
    wtb_t = nc.dram_tensor("wtb", [64, 64], bf16, kind="ExternalInput")
    y_t = nc.dram_tensor("y", [band, C], f32, kind="ExternalOutput")

    cc_in = nc.dram_tensor("cc_in", [2, 64], f32, kind="Internal")
    cc_out = nc.dram_tensor("cc_out", [2, 64], f32, kind="Internal", addr_space="Shared")

    eq = mybir.AluOpType.is_equal
    with tile.TileContext(nc) as tc:
        with (
            tc.tile_pool(name="const", bufs=1) as cp,
            tc.tile_pool(name="tbl", bufs=1) as tp,
            tc.tile_pool(name="dst", bufs=12) as dp,
            tc.tile_pool(name="oh", bufs=6) as ohp,
            tc.tile_pool(name="agg", bufs=1) as aggp,
            tc.tile_pool(name="sq", bufs=4) as sqp,
            tc.tile_pool(name="st", bufs=1) as stp,
            tc.tile_pool(name="out", bufs=4) as outp,
            tc.tile_pool(name="pg", bufs=4, space="PSUM") as pgp,
            tc.tile_pool(name="po", bufs=2, space="PSUM") as pop,
            tc.tile_pool(name="pb2", bufs=1, space="PSUM") as pb2p,
        ):
            iota_sb = cp.tile([128, 128], bf16)
            nc.sync.dma_start(iota_sb[:], iota_t.ap())
            ones_sb = cp.tile([1, 128], bf16)
            nc.sync.dma_start(ones_sb[:], ones_t.ap())
            gamma_sb = cp.tile([64, 1], f32)
            nc.sync.dma_start(gamma_sb[:], gamma_t.ap())
            beta_sb = cp.tile([64, 1], f32)
            nc.sync.dma_start(beta_sb[:], beta_t.ap())
            bvec_sb = cp.tile([1, 64], f32)
            nc.sync.dma_start(bvec_sb[:], bvec_t.ap())
            wt_sb = cp.tile([64, 64], f32)
            nc.sync.dma_start(wt_sb[:], wt_t.ap())
            wtb_sb = cp.tile([64, 64], bf16)
            nc.sync.dma_start(wtb_sb[:], wtb_t.ap())
            recip_sb = cp.tile([64, band], f32)
            nc.sync.dma_start(recip_sb[:], recip_t.ap())
            idx_sb = tp.tile([128, cols], mybir.dt.int32)
            nc.sync.dma_start(idx_sb[:], idx_t.ap())
            tgt_sb = tp.tile([128, cols], f32)
            nc.sync.dma_start(tgt_sb[:], tgt_t.ap())

            aggT = aggp.tile([64, band], bf16)
            sqpart = stp.tile([64, gpc], f32)

            # phase 1: per group, gather + one-hot matmul accumulate
            for g in range(gpc):
                psum_g = pgp.tile([64, 128], f32, tag="pg")
                for t in range(tpg):
                    col = g * tpg + t
                    dst = dp.tile([128, C], bf16, tag="dst")
                    nc.gpsimd.indirect_dma_start(
                        out=dst[:],
                        out_offset=None,
                        in_=x_t.ap(),
                        in_offset=bass.IndirectOffsetOnAxis(
                            ap=idx_sb[:, col : col + 1], axis=0
                        ),
                    )
                    oh = ohp.tile([128, 128], bf16, tag="oh")
                    nc.vector.tensor_scalar(
                        out=oh[:],
                        in0=iota_sb[:],
                        scalar1=tgt_sb[:, col : col + 1],
                        scalar2=None,
                        op0=eq,
                    )
                    nc.tensor.matmul(
                        out=psum_g[:],
                        lhsT=dst[:],
                        rhs=oh[:],
                        start=(t == 0),
                        stop=(t == tpg - 1),
                    )
                nc.vector.tensor_tensor(
                    out=aggT[:, g * 128 : (g + 1) * 128],
                    in0=psum_g[:],
                    in1=recip_sb[:, g * 128 : (g + 1) * 128],
                    op=mybir.AluOpType.mult,
                )
                sq_scr = sqp.tile([64, 128], f32, tag="sq")
                nc.scalar.activation(
                    out=sq_scr[:],
                    in_=aggT[:, g * 128 : (g + 1) * 128],
                    func=mybir.ActivationFunctionType.Square,
                    accum_out=sqpart[:, g : g + 1],
                )

            # BN partial stats -> collective
            s_col = stp.tile([64, 1], f32)
            nc.vector.tensor_reduce(
                out=s_col[:], in_=aggT[:], axis=mybir.AxisListType.X,
                op=mybir.AluOpType.add,
            )
            q_col = stp.tile([64, 1], f32)
            nc.vector.tensor_reduce(
                out=q_col[:], in_=sqpart[:], axis=mybir.AxisListType.X,
                op=mybir.AluOpType.add,
            )
            nc.sync.dma_start(cc_in.ap()[0:1, :], s_col[:, 0:1])
            nc.sync.dma_start(cc_in.ap()[1:2, :], q_col[:, 0:1])
            nc.gpsimd.collective_compute(
                "AllReduce",
                mybir.AluOpType.add,
                ins=[cc_in.ap()],
                outs=[cc_out.ap()],
                replica_groups=[list(range(N_CORES))],
            )
            ssum = stp.tile([64, 1], f32)
            nc.sync.dma_start(ssum[:], cc_out.ap()[0:1, :])
            qsum = stp.tile([64, 1], f32)
            nc.sync.dma_start(qsum[:], cc_out.ap()[1:2, :])

            # BN constants + fold into linear
            inv_n = 1.0 / float(n_nodes_real)
            mean = stp.tile([64, 1], f32)
            nc.vector.tensor_scalar(
                out=mean[:], in0=ssum[:], scalar1=inv_n, scalar2=None,
                op0=mybir.AluOpType.mult,
            )
            e2 = stp.tile([64, 1], f32)
            nc.vector.tensor_scalar(
                out=e2[:], in0=qsum[:], scalar1=inv_n, scalar2=None,
                op0=mybir.AluOpType.mult,
            )
            m2 = stp.tile([64, 1], f32)
            nc.vector.tensor_tensor(
                out=m2[:], in0=mean[:], in1=mean[:], op=mybir.AluOpType.mult
            )
            var = stp.tile([64, 1], f32)
            nc.vector.tensor_tensor(
                out=var[:], in0=e2[:], in1=m2[:], op=mybir.AluOpType.subtract
            )
            vare = stp.tile([64, 1], f32)
            nc.vector.tensor_scalar(
                out=vare[:], in0=var[:], scalar1=BN_EPS, scalar2=None,
                op0=mybir.AluOpType.add,
            )
            sd = stp.tile([64, 1], f32)
            nc.scalar.activation(
                out=sd[:], in_=vare[:], func=mybir.ActivationFunctionType.Sqrt
            )
            rstd = stp.tile([64, 1], f32)
            nc.vector.reciprocal(out=rstd[:], in_=sd[:])
            a_col = stp.tile([64, 1], f32)
            nc.vector.tensor_tensor(
                out=a_col[:], in0=rstd[:], in1=gamma_sb[:], op=mybir.AluOpType.mult
            )
            w2 = stp.tile([64, 64], bf16)
            nc.vector.tensor_scalar(
                out=w2[:], in0=wt_sb[:], scalar1=a_col[:, 0:1], scalar2=None,
                op0=mybir.AluOpType.mult,
            )
            ma = stp.tile([64, 1], f32)
            nc.vector.tensor_tensor(
                out=ma[:], in0=mean[:], in1=a_col[:], op=mybir.AluOpType.mult
            )
            cvec = stp.tile([64, 1], bf16)
            nc.vector.tensor_tensor(
                out=cvec[:], in0=beta_sb[:], in1=ma[:], op=mybir.AluOpType.subtract
            )
            pb2 = pb2p.tile([1, 64], f32)
            nc.tensor.matmul(out=pb2[:], lhsT=cvec[:], rhs=wtb_sb[:], start=True, stop=True)
            b2 = stp.tile([1, 64], bf16)
            nc.vector.tensor_tensor(
                out=b2[:], in0=pb2[:], in1=bvec_sb[:], op=mybir.AluOpType.add
            )

            # phase 2: out = relu(aggT.T @ W2 + b2)
            for g in range(gpc):
                po = pop.tile([128, 64], f32, tag="po")
                nc.tensor.matmul(
                    out=po[:],
                    lhsT=aggT[:, g * 128 : (g + 1) * 128],
                    rhs=w2[:],
                    start=True,
                    stop=False,
                )
                nc.tensor.matmul(
                    out=po[:], lhsT=ones_sb[:], rhs=b2[:], start=False, stop=True
                )
                ot = outp.tile([128, C], f32, tag="ot")
                nc.scalar.activation(
                    out=ot[:], in_=po[:], func=mybir.ActivationFunctionType.Relu
                )
                nc.sync.dma_start(y_t.ap()[g * 128 : (g + 1) * 128, :], ot[:])

    nc.compile()
    return nc


_CACHE = {}


def _get_nc(n_nodes_real, n_nodes_tab, gpc, tpg):
    key = (n_nodes_real, n_nodes_tab, gpc, tpg)
    if key not in _CACHE:
        _CACHE[key] = build_nc(*key)
    return _CACHE[key]


def kernel(x, sources, targets, gamma, beta, W, b, _trace=False):
    return _run(x, sources, targets, gamma, beta, W, b, 49, 16, _trace)


def _run(x, sources, targets, gamma, beta, W, b, gpc, tpg, _trace=False):
    x = np.asarray(x, np.float32)
    sources = np.asarray(sources).astype(np.int32)
    targets = np.asarray(targets).astype(np.int32)
    gamma = np.asarray(gamma, np.float32)
    beta = np.asarray(beta, np.float32)
    W = np.asarray(W, np.float32)
    b = np.asarray(b, np.float32)

    n_nodes = x.shape[0]
    idx_tbl, tgt_tbl, recip_tbl, node_group, node_slot = build_tables(
        x, sources, targets, n_nodes, N_CORES, gpc, tpg
    )

    bf16 = ml_dtypes.bfloat16
    x_bf = x.astype(bf16)
    iota = np.tile(np.arange(128, dtype=np.float32)[None, :], (128, 1)).astype(bf16)
    ones = np.ones((1, 128), bf16)
    wt = np.ascontiguousarray(W.T)
    in_maps = []
    for i in range(N_CORES):
        in_maps.append(
            {
                "x": x_bf,
                "idx": idx_tbl[i],
                "tgt": tgt_tbl[i],
                "recip": recip_tbl[i],
                "iota": iota,
                "ones": ones,
                "gamma": gamma.reshape(64, 1),
                "beta": beta.reshape(64, 1),
                "bvec": b.reshape(1, 64),
                "wt": wt,
                "wtb": wt.astype(bf16),
            }
        )

    nc = _get_nc(n_nodes, n_nodes, gpc, tpg)
    res = bass_utils.run_bass_kernel_spmd(
        nc, in_maps, core_ids=list(range(N_CORES)), trace=_trace
    )

    out = np.empty((n_nodes, C), np.float32)
    nodes = np.arange(n_nodes)
    ncore = node_group // gpc
    npos = (node_group % gpc) * 128 + node_slot
    for i in range(N_CORES):
        sel = ncore == i
        out[nodes[sel]] = res.results[i]["y"][npos[sel]]
    kernel.last_exec_time_ns = res.exec_time_ns
    return out

